# revision 1
# baseline (speedup 1.0000x reference)
"""Trainium2 Bass kernel for nn_DecoderLayer_83554293776404 (8-core SPMD).

Decoder layer: RMSNorm -> GQA attention (RoPE, causal) -> residual ->
RMSNorm -> top-2-of-8 MoE -> residual.

Sharding: tokens 128/core for attention (AllGather for k/v and h2),
expert-parallel MoE (one expert per core), ReduceScatter for the final
combine. All matmuls fp32r. Local-diagonal attention runs before the
AllGather-dependent units; weight DMAs are hoisted for overlap.
"""
"""Host-side input prep + numpy golden model for the 8-core decoder layer.

Sharding (v4):
  - tokens 128/core for attention; per-kv-head 4-q-head batching (N=512 MMs)
  - expert-parallel MoE (dense, weighted by router mask)
  - collectives: AG1 (kT_rope+v), AG2 (h2T+wT), RS x2 (downTw + x2T-placed)
"""
import numpy as np
import ml_dtypes

S, D, H, KV, E, TOPK, F = 1024, 1024, 16, 4, 8, 2, 1024
HD = D // H  # 64
NC = 8
TB = S // NC  # 128 tokens per core
EPS = 1e-5
NEG = -1.0e5  # mask bias
KT = D // 128  # 8 k-tiles
QKD = D + KV * HD  # 1280 = q+k proj dims

AG1_PAY = KV * HD * TB + TB * KV * HD  # kT seg + v seg = 65536
AG2_PAY = (D + E) * TB                 # 1032*128
CAP = 320                              # expert capacity (max load ~280)
CC = (CAP + 127) // 128                # capacity chunks for scatter lhsT
CAPP = CC * 128                        # padded capacity (384)


def prep_inputs(inputs):
    """Full harness inputs -> list of per-core input dicts (numpy, device names)."""
    f32 = np.float32
    x = np.asarray(inputs["x"], f32)
    cos = np.asarray(inputs["cos"], f32)
    sin = np.asarray(inputs["sin"], f32)
    mask = np.asarray(inputs["mask"])
    wq = np.asarray(inputs["wq"], f32)
    wk = np.asarray(inputs["wk"], f32)
    wv = np.asarray(inputs["wv"], f32)
    wo = np.asarray(inputs["wo"], f32)
    w_in = np.asarray(inputs["w_in_norm"], f32)
    w_qn = np.asarray(inputs["w_qnorm"], f32)
    w_kn = np.asarray(inputs["w_knorm"], f32)
    w_post = np.asarray(inputs["w_post_norm"], f32)
    w_gate = np.asarray(inputs["w_gate"], f32)
    up_proj = np.asarray(inputs["up_proj"], f32)
    gate_proj = np.asarray(inputs["gate_proj"], f32)
    down_proj = np.asarray(inputs["down_proj"], f32)

    wqk = np.ascontiguousarray(np.concatenate([wq, wk], axis=1))  # [1024, 1280]
    wqkn_row = np.concatenate([w_qn, w_kn]).reshape(1, QKD)

    per_core = []
    for c in range(NC):
        t0 = c * TB
        cs, sn = cos[t0 : t0 + TB], sin[t0 : t0 + TB]  # [128, 64]
        cosqk = np.ascontiguousarray(np.tile(cs, (1, H + KV)))  # [128, 1280]
        sinqk = np.ascontiguousarray(np.tile(sn, (1, H + KV)))
        # mask-derived structures (see build notes)
        mblk = mask[t0 : t0 + TB, :]  # [128 i, 1024 j]
        full_col = mblk.all(axis=0)
        flags = np.full((S,), NEG, f32)
        flags[np.where(full_col)[0]] = 0.0
        flags[t0 : t0 + TB] = NEG  # own block -> local diag path
        partial = (~full_col) & (mblk.any(axis=0))
        partial[t0 : t0 + TB] = False
        if partial.any():
            raise NotImplementedError("mask has partial columns outside own block")
        flags_sb = np.ascontiguousarray(flags.reshape(NC, TB).T)  # [128 j_loc, 8 slot]
        trildiag = np.ascontiguousarray(mblk[:, t0 : t0 + TB].T.astype(f32))
        sel = np.zeros((TB, S), f32)
        sel[np.arange(TB), t0 + np.arange(TB)] = 1.0
        ec_row = np.zeros((1, E), ml_dtypes.bfloat16)
        ec_row[0, c] = 1.0
        iota_row = np.arange(CAP, dtype=f32).reshape(1, CAP)
        iota_cols = (
            np.arange(128)[:, None] + 128 * np.arange(CC)[None, :]
        ).astype(f32)
        iota_cols[iota_cols >= CAP] = -2.0  # guard: padded slots never match

        d = {
            "x_blk": np.ascontiguousarray(x[t0 : t0 + TB]),
            "wqk": wqk.astype(ml_dtypes.bfloat16),
            "wv_in": wv.astype(ml_dtypes.bfloat16),
            "wo_in": wo.astype(ml_dtypes.bfloat16),
            "upT": np.ascontiguousarray(up_proj[c].T).astype(ml_dtypes.bfloat16),  # [D, F]
            "gateT": np.ascontiguousarray(gate_proj[c].T).astype(ml_dtypes.bfloat16),
            "dnT": np.ascontiguousarray(down_proj[c].T).astype(ml_dtypes.bfloat16),
            "wgate_in": w_gate,
            "w_in_row": w_in.reshape(1, D),
            "w_post_row": w_post.reshape(1, D),
            "wqkn_row": wqkn_row,
            "cosqk": cosqk.astype(ml_dtypes.bfloat16),
            "sinqk": sinqk.astype(ml_dtypes.bfloat16),
            "flags_sb": flags_sb,
            "trildiag": trildiag,
            "trildiag_bf": trildiag.astype(ml_dtypes.bfloat16),
            "sel": sel,
            "ec_row": ec_row,
            "iota_row": iota_row,
            "iota_cols": iota_cols,
        }
        per_core.append(d)
    return per_core


import numpy as np

import concourse.bass as bass
import concourse.bacc as bacc
import concourse.mybir as mybir
import concourse.tile as tile
from concourse.masks import make_identity


F32 = mybir.dt.float32
F32R = mybir.dt.float32r
BF16 = mybir.dt.bfloat16
AX = mybir.AxisListType
ALU = mybir.AluOpType
ACTF = mybir.ActivationFunctionType
RG = [list(range(NC))]
HPK = H // KV  # 4 q heads per kv head


def build(debug=False):
    nc = bacc.Bacc("TRN2", target_bir_lowering=False, num_devices=NC)

    def inp(name, shape, dt=F32R):
        return nc.dram_tensor(name, list(shape), dt, kind="ExternalInput")

    x_blk = inp("x_blk", [TB, D], F32)
    wqk = inp("wqk", [D, QKD], BF16)
    wv_in = inp("wv_in", [D, KV * HD], BF16)
    wo_in = inp("wo_in", [D, D], BF16)
    upT_in = inp("upT", [D, F], BF16)
    gateT_in = inp("gateT", [D, F], BF16)
    dnT_in = inp("dnT", [F, D], BF16)
    wgate_in = inp("wgate_in", [D, E])
    w_in_row = inp("w_in_row", [1, D], F32)
    w_post_row = inp("w_post_row", [1, D], F32)
    wqkn_row = inp("wqkn_row", [1, QKD], F32)
    cosqk_in = inp("cosqk", [TB, QKD], BF16)
    sinqk_in = inp("sinqk", [TB, QKD], BF16)
    flags_in = inp("flags_sb", [TB, NC], F32)
    tril_in = inp("trildiag", [TB, TB], F32R)
    trilbf_in = inp("trildiag_bf", [TB, TB], BF16)
    sel_in = inp("sel", [TB, S])
    ec_in = inp("ec_row", [1, E], BF16)
    iota_row_in = inp("iota_row", [1, CAP], F32)
    iota_cols_in = inp("iota_cols", [128, CC], F32)

    out_d = nc.dram_tensor("out_cols", [S, TB], F32, kind="ExternalOutput")
    dbg = {}
    if debug:
        def dout(name, shape, dt=F32):
            dbg[name] = nc.dram_tensor(name, list(shape), dt, kind="ExternalOutput")
        dout("d_h", [TB, D])
        dout("d_qrope", [TB, D])
        dout("d_krope", [TB, KV * HD])
        dout("d_v", [TB, KV * HD])
        dout("d_attnT", [D, TB])
        dout("d_x2", [TB, D])
        dout("d_h2", [TB, D])
        dout("d_g", [TB, E])
        dout("d_wrt", [TB, E])
        dout("d_rsin", [D, S])

    with tile.TileContext(nc) as tc:
        # ---------- persistent pools ----------
        consts_cm = tc.tile_pool(name="consts", bufs=1)
        consts = consts_cm.__enter__()
        act2_cm = tc.tile_pool(name="act2", bufs=1)
        act2 = act2_cm.__enter__()
        dram_cm = tc.tile_pool(name="dram", bufs=1, space="DRAM")
        dram = dram_cm.__enter__()

        ident_f = consts.tile([128, 128], F32)
        make_identity(nc, ident_f)
        ident = consts.tile([128, 128], F32R)
        nc.vector.tensor_copy(ident[:], ident_f[:])
        ident_bf = consts.tile([128, 128], BF16)
        nc.vector.tensor_copy(ident_bf[:], ident_f[:])

        x2_sb = act2.tile([TB, D], F32R)

        ag1_in = dram.tile([AG1_PAY], BF16)
        ag1_out = dram.tile([NC * AG1_PAY], BF16, addr_space="Shared")
        ag2_in = dram.tile([AG2_PAY], BF16)
        ag2_out = dram.tile([NC * AG2_PAY], BF16, addr_space="Shared")
        rs_in_a = dram.tile([D, S // 2], BF16)
        rs_in_b = dram.tile([D, S // 2], BF16)
        rs_out_a = dram.tile([TB, S // 2], BF16)
        rs_out_b = dram.tile([TB, S // 2], BF16)
        bounce_i = dram.tile([TB, NC], F32)
        bounce_w = dram.tile([TB, NC], F32)

        # attention-lifetime pool (phases 1-4)
        pa_cm = tc.tile_pool(name="pa", bufs=1)
        pa = pa_cm.__enter__()
        # ================= phase 1+2: h, qkv, norms, rope =================
        p1_cm = tc.tile_pool(name="p1", bufs=1)
        p1 = p1_cm.__enter__()
        ps1_cm = tc.tile_pool(name="ps1", bufs=1, space="PSUM")
        ps1 = ps1_cm.__enter__()

        rowbuf = p1.tile([1, D + QKD], F32)
        nc.sync.dma_start(rowbuf[:, 0:D], w_in_row.ap())
        nc.sync.dma_start(rowbuf[:, D:], wqkn_row.ap())
        w_in_b = p1.tile([128, D], F32)
        nc.gpsimd.partition_broadcast(w_in_b[:], rowbuf[:, 0:D])
        wqkn_b = p1.tile([128, QKD], F32)
        nc.gpsimd.partition_broadcast(wqkn_b[:], rowbuf[:, D:])
        cosqk = p1.tile([TB, QKD], BF16)
        nc.sync.dma_start(cosqk[:], cosqk_in.ap())
        sinqk = p1.tile([TB, QKD], BF16)
        nc.sync.dma_start(sinqk[:], sinqk_in.ap())
        flags = pa.tile([TB, NC], F32)
        nc.sync.dma_start(flags[:], flags_in.ap())
        tril = consts.tile([TB, TB], F32R)
        nc.sync.dma_start(tril[:], tril_in.ap())
        tril_bf = consts.tile([TB, TB], BF16)
        nc.sync.dma_start(tril_bf[:], trilbf_in.ap())

        x_sb = pa.tile([TB, D], F32)
        nc.sync.dma_start(x_sb[:], x_blk.ap())
        ssq = p1.tile([TB, 1], F32)
        scratch = p1.tile([TB, D], F32)
        nc.scalar.activation(scratch[:], x_sb[:], ACTF.Square, accum_out=ssq[:])
        rsq = p1.tile([TB, 1], F32)
        nc.vector.tensor_scalar(rsq[:], ssq[:], 1.0 / D, EPS, ALU.mult, ALU.add)
        nc.scalar.sqrt(rsq[:], rsq[:])
        nc.vector.reciprocal(rsq[:], rsq[:])
        h_sb = p1.tile([TB, D], BF16)
        nc.vector.scalar_tensor_tensor(
            h_sb[:], x_sb[:], rsq[:], w_in_b[:], ALU.mult, ALU.mult
        )
        if debug:
            nc.sync.dma_start(dbg["d_h"].ap(), h_sb[:].bitcast(F32))
        hT = p1.tile([128, KT, TB], BF16)
        for k in range(KT):
            tp = ps1.tile([128, 128], BF16, tag="tsp", bufs=2)
            nc.tensor.transpose(tp[:], h_sb[:, 128 * k : 128 * (k + 1)], ident_bf[:])
            nc.vector.tensor_copy(hT[:, k, :], tp[:])

        wqk_sb = p1.tile([128, KT, QKD], BF16)
        nc.scalar.dma_start(wqk_sb[:], wqk.ap().rearrange("(k p) m -> p k m", p=128))
        wv_sb = p1.tile([128, KT, KV * HD], BF16)
        nc.scalar.dma_start(wv_sb[:], wv_in.ap().rearrange("(k p) m -> p k m", p=128))

        # qk projection: 3 psum chunks (512/512/256)
        chunks = [(0, 512), (512, 512), (1024, 256)]
        qk_ps = []
        ssq_parts = []
        for ci, (c0, cw) in enumerate(chunks):
            pq = ps1.tile([TB, cw], F32, tag=f"pq{ci}")
            for k in range(KT):
                nc.tensor.matmul(
                    pq[:], hT[:, k, :], wqk_sb[:, k, c0 : c0 + cw],
                    start=(k == 0), stop=(k == KT - 1),
                )
            qk_ps.append(pq)
            sa = p1.tile([TB, 1], F32, tag=f"sa{ci}")
            nc.scalar.activation(
                scratch[:, 0:cw], pq[:], ACTF.Square, accum_out=sa[:]
            )
            ssq_parts.append(sa)
        # v projection
        pv = ps1.tile([TB, KV * HD], F32, tag="pv")
        for k in range(KT):
            nc.tensor.matmul(
                pv[:], hT[:, k, :], wv_sb[:, k, :], start=(k == 0), stop=(k == KT - 1)
            )
        v_aug_loc = pa.tile([TB, KV, HD + 1], BF16)
        nc.vector.memset(v_aug_loc[:], 1.0)
        nc.vector.tensor_copy(
            v_aug_loc[:, :, 0:HD], pv[:].rearrange("t (kv d) -> t kv d", kv=KV)
        )
        if debug:
            nc.sync.dma_start(
                dbg["d_v"].ap().rearrange("t (kv d) -> t kv d", kv=KV),
                v_aug_loc[:, :, 0:HD].bitcast(F32),
            )

        # norm scales
        ssq_q = p1.tile([TB, 1], F32)
        nc.vector.tensor_add(ssq_q[:], ssq_parts[0][:], ssq_parts[1][:])
        nc.vector.tensor_scalar(ssq_q[:], ssq_q[:], 1.0 / D, EPS, ALU.mult, ALU.add)
        nc.scalar.sqrt(ssq_q[:], ssq_q[:])
        nc.vector.reciprocal(ssq_q[:], ssq_q[:])
        nc.vector.tensor_scalar_mul(ssq_q[:], ssq_q[:], float(HD) ** -0.5)
        ssq_k = p1.tile([TB, 1], F32)
        nc.vector.tensor_scalar(
            ssq_k[:], ssq_parts[2][:], 1.0 / (KV * HD), EPS, ALU.mult, ALU.add
        )
        nc.scalar.sqrt(ssq_k[:], ssq_k[:])
        nc.vector.reciprocal(ssq_k[:], ssq_k[:])

        # normalize (q and k parts) -> qk_n
        qk_n = p1.tile([TB, QKD], BF16)
        for ci, (c0, cw) in enumerate(chunks):
            rs_ap = ssq_q if ci < 2 else ssq_k
            nc.vector.scalar_tensor_tensor(
                qk_n[:, c0 : c0 + cw], qk_ps[ci][:], rs_ap[:],
                wqkn_b[:, c0 : c0 + cw], ALU.mult, ALU.mult,
            )
        # rope: view as [TB, 20, 2, 32]
        qk_v = qk_n[:].rearrange("t (g two h) -> t g two h", two=2, h=HD // 2)
        rot = p1.tile([TB, H + KV, 2, HD // 2], BF16)
        nc.vector.tensor_scalar_mul(rot[:, :, 0, :], qk_v[:, :, 1, :], -1.0)
        nc.vector.tensor_copy(rot[:, :, 1, :], qk_v[:, :, 0, :])
        qk_rope = p1.tile([TB, QKD], BF16)
        nc.vector.tensor_mul(qk_rope[:], qk_n[:], cosqk[:])
        rot_s = p1.tile([TB, QKD], BF16)
        nc.vector.tensor_mul(
            rot_s[:], rot[:].rearrange("t g two h -> t (g two h)"), sinqk[:]
        )
        nc.vector.tensor_add(qk_rope[:], qk_rope[:], rot_s[:])
        if debug:
            nc.sync.dma_start(dbg["d_qrope"].ap(), qk_rope[:, 0:D].bitcast(F32))
            nc.sync.dma_start(dbg["d_krope"].ap(), qk_rope[:, D:QKD].bitcast(F32))

        # transposes: qT_g [64, H, TB], kT_diag [64, KV, TB]
        qT_g = pa.tile([64, H, TB], BF16)
        for h_i in range(H):
            tq = ps1.tile([128, 128], BF16, tag="tsp", bufs=2)
            nc.tensor.transpose(
                tq[0:64, :], qk_rope[:, HD * h_i : HD * (h_i + 1)], ident_bf[:]
            )
            nc.vector.tensor_copy(qT_g[:, h_i, :], tq[0:64, :])
        kT_diag = pa.tile([64, KV, TB], BF16)
        for kv in range(KV):
            tk = ps1.tile([128, 128], BF16, tag="tsp", bufs=2)
            nc.tensor.transpose(
                tk[0:64, :], qk_rope[:, D + HD * kv : D + HD * (kv + 1)], ident_bf[:]
            )
            nc.vector.tensor_copy(kT_diag[:, kv, :], tk[0:64, :])

        # ---------- AG1 ----------
        k_seg = ag1_in[:][0 : KV * HD * TB].rearrange("(d kv t) -> d kv t", kv=KV, d=HD)
        nc.sync.dma_start(k_seg, kT_diag[:])
        v_seg = ag1_in[:][KV * HD * TB :].rearrange("(t kv d) -> t kv d", t=TB, kv=KV)
        nc.sync.dma_start(v_seg, v_aug_loc[:, :, 0:HD])
        nc.gpsimd.collective_compute(
            "AllGather", ALU.bypass, replica_groups=RG,
            ins=[ag1_in[:]], outs=[ag1_out[:]],
        )
        kT_sb = pa.tile([64, NC, KV, TB], BF16)
        v_sb = pa.tile([TB, NC, KV, HD + 1], BF16)
        nc.vector.memset(v_sb[:], 1.0)
        ag1v = ag1_out[:].rearrange("(r x) -> r x", r=NC)
        kpart = ag1v[:, 0 : KV * HD * TB].rearrange(
            "r (d kv t) -> d r kv t", kv=KV, d=HD
        )
        nc.sync.dma_start(kT_sb[:], kpart)
        for r in range(NC):
            vpart = ag1v[r, KV * HD * TB :].rearrange(
                "(t kv d) -> t kv d", t=TB, kv=KV
            )
            nc.scalar.dma_start(v_sb[:, r, :, 0:HD], vpart)

        ps1_cm.__exit__(None, None, None)
        p1_cm.__exit__(None, None, None)
        psa_cm = tc.tile_pool(name="psa", bufs=1, space="PSUM")
        psa = psa_cm.__enter__()
        pwo_cm = tc.tile_pool(name="pwo", bufs=1)
        pwo = pwo_cm.__enter__()
        wo_sb = pwo.tile([128, KT, D], BF16)
        nc.scalar.dma_start(wo_sb[:], wo_in.ap().rearrange("(k p) m -> p k m", p=128))

        # ================= phase 3: attention =================
        attnT = pa.tile([128, KT, TB], BF16)
        oddtmp = pa.tile([64, KT, TB], BF16)
        n_units = NC + 1
        for kv in range(KV):
            o_ps = psa.tile([128, HPK * TB], F32, tag="ops", bufs=2)
            for ui in range(n_units):
                u = NC if ui == 0 else ui - 1  # diag first: overlaps AG1
                is_diag = u == NC
                sc_ps = psa.tile([128, HPK * TB], F32, tag="scps", bufs=3)
                lhs = kT_diag[:, kv, :] if is_diag else kT_sb[:, u, kv, :]
                nc.tensor.matmul(
                    sc_ps[:],
                    lhs,
                    qT_g[:, kv * HPK : (kv + 1) * HPK, :],
                    start=True, stop=True,
                )
                pt = pa.tile([128, HPK * TB], BF16, tag="pt", bufs=3)
                if is_diag:
                    nc.scalar.activation(pt[:], sc_ps[:], ACTF.Exp)
                    ptv = pt[:].rearrange("p (h t) -> p h t", h=HPK)
                    nc.vector.tensor_mul(
                        ptv, ptv, tril_bf[:].unsqueeze(1).broadcast_to([TB, HPK, TB])
                    )
                else:
                    nc.scalar.activation(
                        pt[:], sc_ps[:], ACTF.Exp, bias=flags[:, u : u + 1]
                    )
                vt = v_aug_loc[:, :, :] if is_diag else v_sb[:, u, :, :]
                nc.tensor.matmul(
                    o_ps[0:65, :],
                    vt[:, kv, :],
                    pt[:],
                    start=(ui == 0), stop=(ui == n_units - 1),
                )
            # normalize 4 heads of this kv
            recip = pa.tile([1, HPK * TB], F32, tag="recip", bufs=2)
            nc.vector.reciprocal(recip[:], o_ps[64:65, :])
            rb = pa.tile([64, HPK * TB], F32, tag="rb", bufs=2)
            nc.gpsimd.partition_broadcast(rb[:], recip[:], channels=64)
            for hh in range(HPK):
                h_i = kv * HPK + hh
                m, po = divmod(h_i, 2)
                dst = attnT[0:64, m, :] if po == 0 else oddtmp[:, m, :]
                nc.vector.tensor_mul(
                    dst,
                    o_ps[0:64, TB * hh : TB * (hh + 1)],
                    rb[:, TB * hh : TB * (hh + 1)],
                )
        nc.sync.dma_start(attnT[64:128, :, :], oddtmp[:])
        if debug:
            nc.sync.dma_start(
                dbg["d_attnT"].ap().rearrange("(k p) t -> p k t", p=128),
                attnT[:].bitcast(F32),
            )

        # ================= phase 4: wo + residual =================
        for nn2 in range(2):
            px = psa.tile([TB, 512], F32, tag="px", bufs=2)
            for k in range(KT):
                nc.tensor.matmul(
                    px[:], attnT[:, k, :], wo_sb[:, k, 512 * nn2 : 512 * (nn2 + 1)],
                    start=(k == 0), stop=(k == KT - 1),
                )
            nc.vector.tensor_add(
                x2_sb[:, 512 * nn2 : 512 * (nn2 + 1)],
                px[:],
                x_sb[:, 512 * nn2 : 512 * (nn2 + 1)],
            )
        if debug:
            nc.sync.dma_start(dbg["d_x2"].ap(), x2_sb[:].bitcast(F32))

        pwo_cm.__exit__(None, None, None)
        psa_cm.__exit__(None, None, None)
        pa_cm.__exit__(None, None, None)

        # ================= phase 5: h2, router, AG2 =================
        pm_cm = tc.tile_pool(name="pm", bufs=1)
        pm = pm_cm.__enter__()
        ps5_cm = tc.tile_pool(name="ps5", bufs=1, space="PSUM")
        ps5 = ps5_cm.__enter__()

        rowpost = pm.tile([1, D], F32)
        nc.sync.dma_start(rowpost[:], w_post_row.ap())
        w_post_b = pm.tile([128, D], F32)
        nc.gpsimd.partition_broadcast(w_post_b[:], rowpost[:])
        ec_sb = pm.tile([1, E], BF16)
        nc.sync.dma_start(ec_sb[:], ec_in.ap())
        ecb = pm.tile([128, E], BF16)
        nc.gpsimd.partition_broadcast(ecb[:], ec_sb[:])
        iota_r = pm.tile([1, CAP], F32)
        nc.sync.dma_start(iota_r[:], iota_row_in.ap())
        iota_b = pm.tile([128, CAP], F32)
        nc.gpsimd.partition_broadcast(iota_b[:], iota_r[:])
        iotac = pm.tile([128, CC], F32)
        nc.sync.dma_start(iotac[:], iota_cols_in.ap())
        sel_sb = pm.tile([TB, S], F32R)
        nc.sync.dma_start(sel_sb[:], sel_in.ap())
        upT_w = pm.tile([128, KT, F], BF16)
        gateT_w = pm.tile([128, KT, F], BF16)
        dnT_w = pm.tile([128, KT, D], BF16)
        nc.scalar.dma_start(upT_w[:], upT_in.ap().rearrange("(k p) m -> p k m", p=128))
        nc.scalar.dma_start(
            gateT_w[:], gateT_in.ap().rearrange("(k p) m -> p k m", p=128)
        )
        nc.scalar.dma_start(dnT_w[:], dnT_in.ap().rearrange("(k p) m -> p k m", p=128))
        ssq2 = pm.tile([TB, 1], F32)
        scratch2 = pm.tile([TB, D], F32)
        nc.scalar.activation(scratch2[:], x2_sb[:], ACTF.Square, accum_out=ssq2[:])
        nc.vector.tensor_scalar(ssq2[:], ssq2[:], 1.0 / D, EPS, ALU.mult, ALU.add)
        nc.scalar.sqrt(ssq2[:], ssq2[:])
        nc.vector.reciprocal(ssq2[:], ssq2[:])
        h2_sb = pm.tile([TB, D], BF16)
        nc.vector.scalar_tensor_tensor(
            h2_sb[:], x2_sb[:], ssq2[:], w_post_b[:], ALU.mult, ALU.mult
        )
        if debug:
            pass
        h2T_loc = pm.tile([128, KT, TB], F32R)
        for k in range(KT):
            tp2 = ps5.tile([128, 128], BF16, tag="tp2", bufs=2)
            nc.tensor.transpose(tp2[:], h2_sb[:, 128 * k : 128 * (k + 1)], ident_bf[:])
            nc.vector.tensor_copy(h2T_loc[:, k, :], tp2[:])

        wg_sb = pm.tile([128, KT, E], F32R)
        nc.sync.dma_start(wg_sb[:], wgate_in.ap().rearrange("(k p) e -> p k e", p=128))
        plog = ps5.tile([TB, E], F32, tag="plog")
        for k in range(KT):
            nc.tensor.matmul(
                plog[:], h2T_loc[:, k, :], wg_sb[:, k, :],
                start=(k == 0), stop=(k == KT - 1),
            )
        gmax = pm.tile([TB, 1], F32)
        nc.vector.tensor_reduce(gmax[:], plog[:], AX.X, ALU.max)
        negmax = pm.tile([TB, 1], F32)
        nc.vector.tensor_scalar_mul(negmax[:], gmax[:], -1.0)
        gexp = pm.tile([TB, E], F32)
        gsum = pm.tile([TB, 1], F32)
        nc.scalar.activation(
            gexp[:], plog[:], ACTF.Exp, bias=negmax[:], accum_out=gsum[:]
        )
        grecip = pm.tile([TB, 1], F32)
        nc.vector.reciprocal(grecip[:], gsum[:])
        g_sb = pm.tile([TB, E], F32)
        nc.vector.tensor_scalar_mul(g_sb[:], gexp[:], grecip[:])
        top8 = pm.tile([TB, 8], F32)
        nc.vector.max(top8[:], g_sb[:])
        selm = pm.tile([TB, E], F32)
        nc.vector.tensor_scalar(selm[:], g_sb[:], top8[:, 1:2], None, ALU.is_ge)
        wrt = pm.tile([TB, E], BF16)
        nc.vector.tensor_mul(wrt[:], g_sb[:], selm[:])
        if debug:
            nc.sync.dma_start(dbg["d_g"].ap(), g_sb[:])
            nc.sync.dma_start(dbg["d_wrt"].ap(), wrt[:].bitcast(F32))
        # ---------- AG2: token-major h2 + router weights ----------
        h2seg = ag2_in[:][0 : D * TB].rearrange("(t d) -> t d", t=TB)
        nc.sync.dma_start(h2seg, h2_sb[:])
        wseg = ag2_in[:][D * TB :].rearrange("(t e) -> t e", t=TB)
        nc.sync.dma_start(wseg, wrt[:])
        nc.gpsimd.collective_compute(
            "AllGather", ALU.bypass, replica_groups=RG,
            ins=[ag2_in[:]], outs=[ag2_out[:]],
        )
        h2_full = pm.tile([TB, NC, D], BF16)
        wrt_full = pm.tile([TB, NC, E], BF16)
        ag2v = ag2_out[:].rearrange("(r x) -> r x", r=NC)
        nc.sync.dma_start(
            wrt_full[:], ag2v[:, D * TB :].rearrange("r (t e) -> t r e", t=TB)
        )
        for r in range(NC):
            eng = nc.sync if r % 2 == 0 else nc.scalar
            eng.dma_start(
                h2_full[:, r, :],
                ag2v[r, 0 : D * TB].rearrange("(t d) -> t d", t=TB),
            )

        # ---------- routing compaction: slot index per token ----------
        wtmp = pm.tile([TB, NC, E], F32)
        nc.vector.tensor_mul(
            wtmp[:], wrt_full[:],
            ecb[:].unsqueeze(1).broadcast_to([TB, NC, E]),
        )
        w_sel = pm.tile([TB, NC, 1], F32)
        nc.vector.tensor_reduce(w_sel[:], wtmp[:], AX.X, ALU.add)
        m_r = pm.tile([TB, NC], F32R)
        nc.vector.tensor_scalar(m_r[:], w_sel[:, :, 0], 0.0, None, ALU.is_gt)
        pin_ps = ps5.tile([TB, NC], F32, tag="pin")
        nc.tensor.matmul(pin_ps[:], tril[:], m_r[:], start=True, stop=True)
        # block totals = last row of the inclusive prefix; exclusive cumsum
        # over the 8 blocks via log-step shifted adds on a [1,8] row
        pin_sb = pm.tile([TB, NC], F32)
        nc.vector.tensor_copy(pin_sb[:], pin_ps[:])
        tot_row = pm.tile([1, NC], F32)
        nc.scalar.dma_start(tot_row[:], pin_sb[127:128, :])
        c1 = pm.tile([1, NC], F32)
        nc.vector.tensor_copy(c1[:, 0:1], tot_row[:, 0:1])
        nc.vector.tensor_add(c1[:, 1:NC], tot_row[:, 1:NC], tot_row[:, 0 : NC - 1])
        c2 = pm.tile([1, NC], F32)
        nc.vector.tensor_copy(c2[:, 0:2], c1[:, 0:2])
        nc.vector.tensor_add(c2[:, 2:NC], c1[:, 2:NC], c1[:, 0 : NC - 2])
        c3 = pm.tile([1, NC], F32)
        nc.vector.tensor_copy(c3[:, 0:4], c2[:, 0:4])
        nc.vector.tensor_add(c3[:, 4:NC], c2[:, 4:NC], c2[:, 0 : NC - 4])
        offrow = pm.tile([1, NC], F32)
        nc.vector.tensor_sub(offrow[:], c3[:], tot_row[:])
        off_b = pm.tile([128, NC], F32)
        nc.gpsimd.partition_broadcast(off_b[:], offrow[:])
        rank = pm.tile([TB, NC], F32)
        nc.vector.tensor_add(rank[:], pin_ps[:], off_b[:])
        idx = pm.tile([TB, NC], F32)
        nc.vector.tensor_mul(idx[:], rank[:], m_r[:].bitcast(F32))
        nc.vector.tensor_scalar(idx[:], idx[:], 1.0, -1.0, ALU.mult, ALU.add)

        # Gt [t, r, cap]: one-hot gather matrix (token-major rhs)
        Gt = pm.tile([TB, NC, CAP], BF16)
        for r in range(NC):
            nc.vector.tensor_scalar(
                Gt[:, r, :], iota_b[:], idx[:, r : r + 1], None,
                ALU.is_equal,
            )
        ps5_cm.__exit__(None, None, None)
        ps6_cm = tc.tile_pool(name="ps6", bufs=1, space="PSUM")
        psm = ps6_cm.__enter__()

        # ================= phase 6: sparse MoE over CAP slots =================
        h2gT = pm.tile([128, KT, CAP], BF16)
        for k in range(KT):
            gps = psm.tile([128, CAP], F32, tag="gps", bufs=2)
            for r in range(NC):
                nc.tensor.matmul(
                    gps[:], h2_full[:, r, 128 * k : 128 * (k + 1)], Gt[:, r, :],
                    start=(r == 0), stop=(r == NC - 1),
                )
            nc.vector.tensor_copy(h2gT[:, k, :], gps[:])

        hidT = pm.tile([128, KT, CAP], BF16)
        for ft in range(KT):
            pu = psm.tile([128, CAP], F32, tag="pu", bufs=2)
            pg = psm.tile([128, CAP], F32, tag="pg", bufs=2)
            for k in range(KT):
                nc.tensor.matmul(
                    pu[:], upT_w[:, k, 128 * ft : 128 * (ft + 1)], h2gT[:, k, :],
                    start=(k == 0), stop=(k == KT - 1),
                )
            for k in range(KT):
                nc.tensor.matmul(
                    pg[:], gateT_w[:, k, 128 * ft : 128 * (ft + 1)], h2gT[:, k, :],
                    start=(k == 0), stop=(k == KT - 1),
                )
            sg = pm.tile([128, CAP], F32, tag="sg", bufs=2)
            nc.scalar.activation(sg[:], pg[:], ACTF.Silu)
            nc.vector.tensor_mul(hidT[:, ft, :], sg[:], pu[:])

        dn_sb = pm.tile([128, KT, CAPP], BF16)
        if CAPP > CAP:
            nc.vector.memset(dn_sb[:, :, CAP:CAPP], 0.0)
        for m in range(KT):
            dps = psm.tile([128, CAP], F32, tag="dn", bufs=2)
            for ft in range(KT):
                nc.tensor.matmul(
                    dps[:], dnT_w[:, ft, 128 * m : 128 * (m + 1)], hidT[:, ft, :],
                    start=(ft == 0), stop=(ft == KT - 1),
                )
            nc.vector.tensor_copy(dn_sb[:, m, 0:CAP], dps[:])

        ps6_cm.__exit__(None, None, None)
        ps6b_cm = tc.tile_pool(name="ps6b", bufs=1, space="PSUM")
        psm = ps6b_cm.__enter__()

        # idx/w as rows (DRAM bounce) for scatter-side Gw
        nc.scalar.dma_start(bounce_i[:], idx[:])
        w_sel_f = pm.tile([TB, NC], F32)
        nc.vector.tensor_copy(w_sel_f[:], w_sel[:, :, 0])
        nc.scalar.dma_start(bounce_w[:], w_sel_f[:])
        idx_row = pm.tile([1, S], F32)
        nc.scalar.dma_start(
            idx_row[:].rearrange("o (r t) -> o r t", r=NC),
            bounce_i[:].rearrange("t r -> r t"),
        )
        w_row = pm.tile([1, S], F32)
        nc.scalar.dma_start(
            w_row[:].rearrange("o (r t) -> o r t", r=NC),
            bounce_w[:].rearrange("t r -> r t"),
        )
        idx_bb = pm.tile([128, S], F32)
        nc.gpsimd.partition_broadcast(idx_bb[:], idx_row[:])
        w_bb = pm.tile([128, S], F32)
        nc.gpsimd.partition_broadcast(w_bb[:], w_row[:])
        Gw = pm.tile([128, CC, S], BF16)
        for cc in range(CC):
            nc.vector.scalar_tensor_tensor(
                Gw[:, cc, :], idx_bb[:], iotac[:, cc : cc + 1],
                w_bb[:], ALU.is_equal, ALU.mult,
            )


        dngT = pm.tile([128, CC, KT, 128], BF16)
        for m in range(KT):
            for cc in range(CC):
                dtt = psm.tile([128, 128], BF16, tag="dtt", bufs=2)
                nc.tensor.transpose(
                    dtt[:], dn_sb[:, m, 128 * cc : 128 * (cc + 1)], ident_bf[:]
                )
                nc.vector.tensor_copy(dngT[:, cc, m, :], dtt[:])

        for half, (rs_in, rs_out) in enumerate(
            [(rs_in_a, rs_out_a), (rs_in_b, rs_out_b)]
        ):
            tsl = slice(512 * half, 512 * (half + 1))
            for m in range(KT):
                sps = psm.tile(
                    [128, 512], F32, tag=f"sps{m % 4}", name=f"sps{half}_{m}"
                )
                for cc in range(CC):
                    nc.tensor.matmul(
                        sps[:], dngT[:, cc, m, :], Gw[:, cc, tsl],
                        start=(cc == 0), stop=False,
                    )
                nc.tensor.matmul(
                    sps[:], x2_sb[:, 128 * m : 128 * (m + 1)], sel_sb[:, tsl],
                    start=False, stop=True,
                )
                osb = pm.tile([128, 512], BF16, tag="osb", bufs=2, name=f"osb{half}_{m}")
                nc.vector.tensor_copy(osb[:], sps[:])
                eng = nc.sync if m % 2 == 0 else nc.scalar
                eng.dma_start(rs_in[:][128 * m : 128 * (m + 1), :], osb[:])
            nc.gpsimd.collective_compute(
                "ReduceScatter", ALU.add, replica_groups=RG,
                ins=[rs_in[:]], outs=[rs_out[:]],
            )

        ps6b_cm.__exit__(None, None, None)
        ps7_cm = tc.tile_pool(name="ps7", bufs=1, space="PSUM")
        psm = ps7_cm.__enter__()

        # ================= phase 7: final =================
        fin = pm.tile([TB, S], BF16)
        nc.sync.dma_start(fin[:, 0:512], rs_out_a[:])
        nc.sync.dma_start(fin[:, 512:1024], rs_out_b[:])
        fout = pm.tile([128, KT, 128], F32)
        for tt in range(KT):
            ftp = psm.tile([128, 128], BF16, tag="ftp", bufs=2)
            nc.tensor.transpose(ftp[:], fin[:, 128 * tt : 128 * (tt + 1)], ident_bf[:])
            nc.vector.tensor_copy(fout[:, tt, :], ftp[:])
        nc.sync.dma_start(
            out_d.ap().rearrange("(k p) t -> p k t", p=128), fout[:]
        )

        ps7_cm.__exit__(None, None, None)
        pm_cm.__exit__(None, None, None)
        dram_cm.__exit__(None, None, None)
        act2_cm.__exit__(None, None, None)
        consts_cm.__exit__(None, None, None)

    nc.compile()
    return nc


_CACHED = {}


def kernel(**inputs):
    import numpy as np
    from concourse.bass_utils import run_bass_kernel_spmd

    per_core = prep_inputs(inputs)
    if "nc" not in _CACHED:
        _CACHED["nc"] = build(debug=False)
    nc = _CACHED["nc"]
    res = run_bass_kernel_spmd(nc, per_core, core_ids=list(range(NC)), trace=False)
    out = np.concatenate([res.results[c]["out_cols"] for c in range(NC)], axis=1)
    return out.astype(np.float32)

